# revision 17
# baseline (speedup 1.0000x reference)
"""Trainium2 Bass kernel for nn_AttentionDecoder (attention + GRU decoder, 22 steps).

Sharding: data-parallel over batch B=32 across 8 NeuronCores (4 batch rows per
core); all weights replicated; the 22-step scan runs locally per core with x and
xW resident in SBUF (no HBM re-reads of x).

Per-core per-step dataflow (all big matmuls in bf16, fp32 PSUM accumulation):
  hWh^T [A,4]   = Wh^T @ h^T                       (PE, 2 k-chunk MMs)
  tanh_b [A,T]  = tanh(xW^T[:, b] + hWh^T[:, b])   (ACT, per-partition bias)
  e^T [128,16]  = tanh-chunk^T @ v per t-chunk     (PE, 16 MMs, tanh as lhsT;
                  lands partition-distributed so softmax needs no DMA)
  att_b         = exp(e^T)  (+accum row sums)      (ACT psum->sbuf, bf16 out)
  ctx_b [1,256] = sum_c att[:,c]^T @ x_chunk(b,c)  (PE, 16 accumulating MMs)
  ctx rows gathered to [4,256] via SBUF->SBUF DMA (cross-partition move)
  softmax denom: ones-matmul -> [4,1], reciprocal; ctx scaled per-partition
  GRU: gi/gh via ctx^T/h^T as stationary [128,4] operands; gates on DVE/ACT
       (sigmoid computed as 0.5 + 0.5*tanh(x/2): keeps ACT in one table set)
  logits [4,C]  = h_new^T.T @ W_cls^T + b_cls      (PE, 9 N-slices, DMA to HBM)
"""
import os
import sys

import numpy as np

os.environ.setdefault("MYCRO_LOCAL_CACHE", "1")
for p in ("/opt/trn_rl_repo",):
    if p not in sys.path and os.path.isdir(p):
        sys.path.insert(0, p)

import ml_dtypes  # noqa: E402

import concourse.bass as bass  # noqa: E402
from concourse import bacc  # noqa: E402
import concourse.mybir as mybir  # noqa: E402
import concourse.tile as tile  # noqa: E402
from concourse.alu_op_type import AluOpType  # noqa: E402
from concourse.bass_utils import run_bass_kernel_spmd  # noqa: E402
from concourse.masks import make_identity  # noqa: E402

B, T, D = 32, 2048, 256
H = 256
A = 128
C = 4367
STEPS = 22
NCORES = 8
B4 = B // NCORES          # 4 batch rows per core
KC = D // 128             # 2 contraction chunks of 128
TC = T // 128             # 16 t-chunks per batch row
BT = B4 * T               # 8192

F32 = mybir.dt.float32
BF16 = mybir.dt.bfloat16
ACT_F = mybir.ActivationFunctionType

_BUILt = {}


def _cls_slices():
    out = []
    n0 = 0
    while n0 < C:
        nn = min(512, C - n0)
        out.append((n0, nn))
        n0 += nn
    return out


def build_nc() -> bass.Bass:
    nc = bacc.Bacc()

    x_nat = nc.declare_dram_parameter("x_nat", [128, B4 * TC, D], BF16, isOutput=False)
    xT = nc.declare_dram_parameter("xT", [128, KC, BT], BF16, isOutput=False)
    wx = nc.declare_dram_parameter("wx", [128, KC, A], BF16, isOutput=False)
    wh = nc.declare_dram_parameter("wh", [128, KC, A], BF16, isOutput=False)
    v = nc.declare_dram_parameter("v", [128, 1], BF16, isOutput=False)
    wihT = nc.declare_dram_parameter("wihT", [128, KC, 3 * H], BF16, isOutput=False)
    whhT = nc.declare_dram_parameter("whhT", [128, KC, 3 * H], BF16, isOutput=False)
    wclsT = nc.declare_dram_parameter("wclsT", [128, KC, C], BF16, isOutput=False)
    bias_cat = nc.declare_dram_parameter("bias_cat", [B4, 4 * H], F32, isOutput=False)
    out_ext = nc.declare_dram_parameter("out", [STEPS, B4, C], F32, isOutput=True)

    with tile.TileContext(nc) as tc:
        with tc.tile_pool(name="singles", bufs=1) as singles:
            x_sb = singles.tile([128, B4 * TC, D], BF16, tag="x_sb")
            nc.gpsimd.dma_start(out=x_sb[:], in_=x_nat[:])
            xw_sb = singles.tile([128, BT], BF16, tag="xw_sb")
            wih_sb = singles.tile([128, KC, 3 * H], BF16, tag="wih_sb")
            nc.scalar.dma_start(out=wih_sb[:], in_=wihT[:])
            whh_sb = singles.tile([128, KC, 3 * H], BF16, tag="whh_sb")
            nc.scalar.dma_start(out=whh_sb[:], in_=whhT[:])
            wcls_sb = singles.tile([128, KC, C], BF16, tag="wcls_sb")
            nc.scalar.dma_start(out=wcls_sb[:], in_=wclsT[:])
            wh_sb = singles.tile([128, KC, A], BF16, tag="wh_sb")
            nc.default_dma_engine.dma_start(out=wh_sb[:], in_=wh[:])
            v_sb = singles.tile([128, 1], BF16, tag="v_sb")
            nc.default_dma_engine.dma_start(out=v_sb[:], in_=v[:])
            bias_sb = singles.tile([B4, 4 * H], F32, tag="bias_sb")
            nc.default_dma_engine.dma_start(out=bias_sb[:], in_=bias_cat[:])
            ones_sb = singles.tile([128, 1], F32, tag="ones_sb")
            nc.vector.memset(ones_sb[:], 1.0)
            ident = singles.tile([128, 128], F32, tag="ident")
            make_identity(nc, ident[:])

            h0 = singles.tile([B4, H], F32, tag="h0")
            nc.vector.memset(h0[:], 0.0)
            hT0 = singles.tile([128, KC, B4], BF16, tag="hT0")
            nc.vector.memset(hT0[:], 0.0)
            hwh0 = singles.tile([128, B4], F32, tag="hwh0")
            nc.vector.memset(hwh0[:], 0.0)

            # ---- startup: xW^T = Wx^T @ x^T, stored bf16, then xT freed ----
            with (
                tc.tile_pool(name="xt_pool", bufs=1) as xtp,
                tc.tile_pool(name="xw_ps", bufs=2, space="PSUM") as xwps,
            ):
                xT_sb = xtp.tile([128, KC, BT], BF16, tag="xT_sb")
                nc.default_dma_engine.dma_start(out=xT_sb[:], in_=xT[:])
                wx_sb = xtp.tile([128, KC, A], BF16, tag="wx_sb")
                nc.default_dma_engine.dma_start(out=wx_sb[:], in_=wx[:])
                for nck in range(BT // 512):
                    ps = xwps.tile([128, 512], F32, tag="xw")
                    sl = slice(512 * nck, 512 * (nck + 1))
                    nc.tensor.matmul(ps[:], wx_sb[:, 0, :], xT_sb[:, 0, sl],
                                     start=True, stop=False)
                    nc.tensor.matmul(ps[:], wx_sb[:, 1, :], xT_sb[:, 1, sl],
                                     start=False, stop=True)
                    nc.vector.tensor_copy(xw_sb[:, sl], ps[:])

            # ---- steady-state pools ----
            with (
                tc.tile_pool(name="tan_pool", bufs=2) as tan_pool,
                tc.tile_pool(name="att_pool", bufs=3) as att_pool,
                tc.tile_pool(name="work", bufs=2) as work,
                tc.tile_pool(name="e_ps", bufs=2, space="PSUM") as e_ps_pool,
                tc.tile_pool(name="ctx_ps", bufs=1, space="PSUM") as ctx_ps_pool,
                tc.tile_pool(name="g_ps", bufs=1, space="PSUM") as g_ps_pool,
                tc.tile_pool(name="logit_ps", bufs=2, space="PSUM") as logit_ps_pool,
                tc.tile_pool(name="small_ps", bufs=1, space="PSUM") as small_ps,
            ):
                h_prev, hT_prev, hwh_sb = h0, hT0, hwh0

                for s in range(STEPS):
                    accum = work.tile([128, B4], F32, tag="accum")
                    # ctx rows collected on partition 0 (per-b segments)
                    ctx_cat = work.tile([1, B4 * H], F32, tag="ctx_cat")

                    def flush_b(b, e_ps, accum=accum, ctx_cat=ctx_cat):
                        att = att_pool.tile([128, TC], BF16, tag="att")
                        nc.scalar.activation(att[:], e_ps[:], ACT_F.Exp,
                                             accum_out=accum[:, b:b + 1])
                        ctx_ps = ctx_ps_pool.tile([1, H], F32, tag="ctx")
                        for c in range(TC):
                            nc.tensor.matmul(ctx_ps[:], att[:, c:c + 1],
                                             x_sb[:, b * TC + c, :],
                                             start=(c == 0), stop=(c == TC - 1))
                        nc.vector.tensor_copy(ctx_cat[:, b * H:(b + 1) * H],
                                              ctx_ps[:])

                    pend = None
                    for b in range(B4):
                        tan = tan_pool.tile([128, T], BF16, tag="tan")
                        e_ps = e_ps_pool.tile([128, TC], F32, tag="e")
                        if b < B4 - 1:
                            nc.scalar.activation(tan[:], xw_sb[:, b * T:(b + 1) * T],
                                                 ACT_F.Tanh, bias=hwh_sb[:, b:b + 1])
                            for c in range(TC):
                                nc.tensor.matmul(e_ps[:, c:c + 1],
                                                 tan[:, 128 * c:128 * (c + 1)],
                                                 v_sb[:], start=True, stop=True)
                            if pend is not None:
                                flush_b(*pend)
                        else:
                            # last batch row: halves; previous row's softmax/ctx
                            # is emitted between the halves so ctx_2 overlaps
                            hh = T // 2
                            nc.scalar.activation(tan[:, :hh],
                                                 xw_sb[:, b * T:b * T + hh],
                                                 ACT_F.Tanh, bias=hwh_sb[:, b:b + 1])
                            for c in range(TC // 2):
                                nc.tensor.matmul(e_ps[:, c:c + 1],
                                                 tan[:, 128 * c:128 * (c + 1)],
                                                 v_sb[:], start=True, stop=True)
                            if pend is not None:
                                flush_b(*pend)
                            nc.scalar.activation(tan[:, hh:],
                                                 xw_sb[:, b * T + hh:(b + 1) * T],
                                                 ACT_F.Tanh, bias=hwh_sb[:, b:b + 1])
                            for c in range(TC // 2, TC):
                                nc.tensor.matmul(e_ps[:, c:c + 1],
                                                 tan[:, 128 * c:128 * (c + 1)],
                                                 v_sb[:], start=True, stop=True)
                        pend = (b, e_ps)
                    flush_b(*pend)

                    # softmax denominators as a [1,4] row on partition 0;
                    # ctxT via outer-product matmuls (K=1) with rhs = 1/sum_b,
                    # folding the softmax normalization into the transpose.
                    sums_ps = small_ps.tile([1, B4], F32, tag="small")
                    nc.tensor.matmul(sums_ps[:], ones_sb[:], accum[:],
                                     start=True, stop=True)
                    recip_row = work.tile([1, B4], F32, tag="recip_row")
                    nc.vector.reciprocal(recip_row[:], sums_ps[:])

                    ctxT_ps = small_ps.tile([128, KC, B4], F32, tag="small")
                    for b in range(B4):
                        for kc in range(KC):
                            nc.tensor.matmul(
                                ctxT_ps[:, kc, b:b + 1],
                                ctx_cat[:, b * H + 128 * kc:b * H + 128 * (kc + 1)],
                                recip_row[:, b:b + 1],
                                start=True, stop=True)
                    ctxT = work.tile([128, KC, B4], BF16, tag="ctxT")
                    nc.vector.tensor_copy(ctxT[:], ctxT_ps[:])

                    # GRU gate matmuls: [0:512]=i_rz+h_rz, [512:768]=i_n, [768:1024]=h_n
                    g_ps = g_ps_pool.tile([B4, 4 * H], F32, tag="g")
                    nc.tensor.matmul(g_ps[:, 0:512], ctxT[:, 0, :], wih_sb[:, 0, 0:512],
                                     start=True, stop=False)
                    nc.tensor.matmul(g_ps[:, 0:512], ctxT[:, 1, :], wih_sb[:, 1, 0:512],
                                     start=False, stop=False)
                    nc.tensor.matmul(g_ps[:, 0:512], hT_prev[:, 0, :], whh_sb[:, 0, 0:512],
                                     start=False, stop=False)
                    nc.tensor.matmul(g_ps[:, 0:512], hT_prev[:, 1, :], whh_sb[:, 1, 0:512],
                                     start=False, stop=True)
                    nc.tensor.matmul(g_ps[:, 512:768], ctxT[:, 0, :], wih_sb[:, 0, 512:768],
                                     start=True, stop=False)
                    nc.tensor.matmul(g_ps[:, 512:768], ctxT[:, 1, :], wih_sb[:, 1, 512:768],
                                     start=False, stop=True)
                    nc.tensor.matmul(g_ps[:, 768:1024], hT_prev[:, 0, :], whh_sb[:, 0, 512:768],
                                     start=True, stop=False)
                    nc.tensor.matmul(g_ps[:, 768:1024], hT_prev[:, 1, :], whh_sb[:, 1, 512:768],
                                     start=False, stop=True)

                    # gates, latency-ordered: r-chain first, z off-chain.
                    # sigmoid(x) = 0.5 + 0.5*tanh(x/2); folded as
                    # r*hn = 0.5*(tanh+1)*hn via scalar_tensor_tensor
                    g_sb = work.tile([B4, 4 * H], F32, tag="g_sb")
                    nc.vector.tensor_add(g_sb[:, 0:H], g_ps[:, 0:H],
                                         bias_sb[:, 0:H])
                    tr_t = work.tile([B4, H], F32, tag="tr_t")
                    nc.scalar.activation(tr_t[:], g_sb[:, 0:H], ACT_F.Tanh,
                                         scale=0.5)
                    nc.vector.tensor_add(g_sb[:, 2 * H:4 * H],
                                         g_ps[:, 2 * H:4 * H],
                                         bias_sb[:, 2 * H:4 * H])
                    rhn = work.tile([B4, H], F32, tag="rhn")
                    nc.vector.scalar_tensor_tensor(
                        rhn[:], tr_t[:], 1.0, g_sb[:, 3 * H:4 * H],
                        AluOpType.add, AluOpType.mult)
                    narg = work.tile([B4, H], F32, tag="narg")
                    nc.vector.scalar_tensor_tensor(
                        narg[:], rhn[:], 0.5, g_sb[:, 2 * H:3 * H],
                        AluOpType.mult, AluOpType.add)
                    nt = work.tile([B4, H], F32, tag="nt")
                    nc.scalar.activation(nt[:], narg[:], ACT_F.Tanh)
                    nc.vector.tensor_add(g_sb[:, H:2 * H], g_ps[:, H:2 * H],
                                         bias_sb[:, H:2 * H])
                    tz_t = work.tile([B4, H], F32, tag="tz_t")
                    nc.scalar.activation(tz_t[:], g_sb[:, H:2 * H], ACT_F.Tanh,
                                         scale=0.5)
                    dd = work.tile([B4, H], F32, tag="dd")
                    nc.vector.tensor_sub(dd[:], h_prev[:], nt[:])
                    nc.vector.scalar_tensor_tensor(
                        dd[:], tz_t[:], 1.0, dd[:],
                        AluOpType.add, AluOpType.mult)
                    h_new = work.tile([B4, H], F32, tag="h")
                    nc.vector.scalar_tensor_tensor(
                        h_new[:], dd[:], 0.5, nt[:],
                        AluOpType.mult, AluOpType.add)

                    hTn = work.tile([128, KC, B4], BF16, tag="hT")
                    for kc in range(KC):
                        tr_ps = small_ps.tile([128, B4], F32, tag="small")
                        nc.tensor.transpose(tr_ps[:],
                                            h_new[:, 128 * kc:128 * (kc + 1)],
                                            ident[:B4, :B4])
                        nc.vector.tensor_copy(hTn[:, kc, :], tr_ps[:])

                    # next step's hWh^T before logits so ACT unblocks early
                    hwh_next = hwh_sb
                    if s + 1 < STEPS:
                        hwh_next = work.tile([128, B4], F32, tag="hwh_sb")
                        hwh_ps = small_ps.tile([128, B4], F32, tag="small")
                        nc.tensor.matmul(hwh_ps[:], wh_sb[:, 0, :], hTn[:, 0, :],
                                         start=True, stop=False)
                        nc.tensor.matmul(hwh_ps[:], wh_sb[:, 1, :], hTn[:, 1, :],
                                         start=False, stop=True)
                        nc.vector.tensor_copy(hwh_next[:], hwh_ps[:])

                    # logits for this step: psum slices -> one staging tile ->
                    # single DMA (b_cls is added host-side)
                    lstage = work.tile([B4, C], F32, tag="lstage")
                    for i, (n0, nn) in enumerate(_cls_slices()):
                        lp = logit_ps_pool.tile([B4, 512], F32, tag="lp")
                        nc.tensor.matmul(lp[:, :nn], hTn[:, 0, :],
                                         wcls_sb[:, 0, n0:n0 + nn],
                                         start=True, stop=False)
                        nc.tensor.matmul(lp[:, :nn], hTn[:, 1, :],
                                         wcls_sb[:, 1, n0:n0 + nn],
                                         start=False, stop=True)
                        nc.vector.tensor_copy(lstage[:, n0:n0 + nn],
                                              lp[:, :nn])
                    nc.gpsimd.dma_start(out=out_ext[s], in_=lstage[:])

                    h_prev, hT_prev, hwh_sb = h_new, hTn, hwh_next
    nc.compile()
    return nc


def _prep_core_inputs(x4, Wx, Wh, v, W_ih, W_hh, b_ih, b_hh, W_cls, b_cls):
    BF = ml_dtypes.bfloat16
    # x_nat[p, b*16 + c, d] = x4[b, t, d] with t = 128*c + p
    xr = x4.reshape(B4, TC, 128, D)              # [b, c, p, d]
    x_nat = np.ascontiguousarray(
        xr.transpose(2, 0, 1, 3).reshape(128, B4 * TC, D)).astype(BF)
    # xT[p, kc, b*T + t] = x4[b, t, kc*128 + p]
    xT = np.ascontiguousarray(
        x4.transpose(2, 0, 1).reshape(KC, 128, BT).transpose(1, 0, 2)).astype(BF)

    def kchunk(w):  # [256, M] -> [128, KC, M]
        return np.ascontiguousarray(
            w.reshape(KC, 128, w.shape[1]).transpose(1, 0, 2)).astype(BF)

    wx_ = kchunk(Wx)                              # [256,128] -> [128,2,128]
    wh_ = kchunk(Wh)
    wihT = kchunk(W_ih.T)                         # [256,768] -> [128,2,768]
    whhT = kchunk(W_hh.T)
    wclsT = kchunk(W_cls.T)                       # [256,4367]-> [128,2,4367]
    v_ = v.reshape(128, 1).astype(BF)
    b_rz = (b_ih[:512] + b_hh[:512]).astype(np.float32)
    bias_cat = np.tile(np.concatenate(
        [b_rz, b_ih[512:].astype(np.float32), b_hh[512:].astype(np.float32)]),
        (B4, 1)).astype(np.float32)
    return {
        "x_nat": x_nat, "xT": xT, "wx": wx_, "wh": wh_, "v": v_,
        "wihT": wihT, "whhT": whhT, "wclsT": wclsT,
        "bias_cat": np.ascontiguousarray(bias_cat),
    }


def _get_nc():
    if "nc" not in _BUILt:
        _BUILt["nc"] = build_nc()
    return _BUILt["nc"]


def _make_in_maps(inputs):
    x = np.asarray(inputs["x"], dtype=np.float32)
    args = {k: np.asarray(inputs[k], dtype=np.float32)
            for k in ("Wx", "Wh", "v", "W_ih", "W_hh", "b_ih", "b_hh",
                      "W_cls", "b_cls")}
    in_maps = []
    for core in range(NCORES):
        x4 = x[core * B4:(core + 1) * B4]
        in_maps.append(_prep_core_inputs(x4, **args))
    return in_maps


def run(inputs, trace=False):
    nc = _get_nc()
    in_maps = _make_in_maps(inputs)
    res = run_bass_kernel_spmd(nc, in_maps, core_ids=list(range(NCORES)),
                               trace=trace)
    out = np.concatenate([np.asarray(res.results[i]["out"], dtype=np.float32)
                          .transpose(1, 0, 2) for i in range(NCORES)], axis=0)
    out += np.asarray(inputs["b_cls"], dtype=np.float32)[None, None, :]
    return out, res


def kernel(**inputs) -> np.ndarray:
    out, _ = run(inputs, trace=False)
    return out
